# revision 10
# baseline (speedup 1.0000x reference)
"""CDR-aligned conditioner kernel for Trainium2 (8 NeuronCores).

Strategy
--------
The reference projects every text token through a 2-layer MLP
(3584 -> 768 -> SiLU -> 384) and then, per (chain_type, cdr_type) pair,
copies the k-th masked text row to the k-th masked protein position.
Only ~2450 of the 16384 text rows are ever read, so:

1. (host) compute the aligned (batch, text_src, protein_dst) triples
   with cheap integer ops — exactly the reference's cumsum/rank logic;
2. (host) gather just those text rows;
3. (device, 8 cores data-parallel over rows) dense MLP on the gathered
   rows:  Y^T = (scale*W2) @ silu(W1 @ X^T + b1) + scale*b2;
4. (host) scatter the projected rows into the zero-initialized output.

Device kernel notes (v2 — DMA-issue-bound fix):
- the v1 kernel issued 62 per-tile DMAs; each DMA_DIRECT2D occupies the
  sync engine ~650 ns generating descriptors, so the whole kernel was
  paced by DMA *issue* (~40 us serial), not HBM bandwidth.  v2 packs
  every operand partition-major in DRAM ([128, bytes] with large
  per-partition contiguous runs) so a handful of big DMAs move the same
  data: x on the scalar-engine HWDGE ring, w1 on the sync ring,
  chunked k-major so the GEMM1 k-loop streams as chunks land.
- all matmul operands are bf16 (full PE rate, half the HBM traffic of
  f32r; ~3e-3 max rel err vs the fp32 reference, tolerance is 2e-2).
- GEMM1 runs k-outer/h-inner with 6 PSUM banks accumulating; GEMM2
  h-outer/c-inner reuses 3 of those banks (tag-aliased) as silu frees
  them.  LDWEIGHTS is hidden by the PE's reorder window; warm matmuls
  stream at ~N/2.4GHz.
- output is written bf16 and upcast on host (halves the tail DMA).
"""

import os
import sys

sys.path.insert(0, "/opt/trn_rl_repo")

import ml_dtypes
import numpy as np

import concourse.bass as bass
import concourse.mybir as mybir
from concourse import bacc
from concourse.bass_utils import run_bass_kernel_spmd
from concourse.tile import TileContext

# Problem constants (hardcoded per contract)
B, L_TEXT, N_TOKEN = 8, 2048, 2048
C_TEXT, C_OUT = 3584, 384
C_HID = C_OUT * 2
CHAIN_TYPES = (1, 2)
CDR_TYPES = (2, 4, 6)
N_CORES = 8

KT = C_TEXT // 128   # 28 k-tiles (contraction of GEMM1)
HT = C_HID // 128    # 6 h-tiles
OT = C_OUT // 128    # 3 out-tiles

F32 = mybir.dt.float32
AF = mybir.ActivationFunctionType

# matmul element type: bf16 (default), float32r, or float32 (exact)
_MM_DT_NAME = os.environ.get("CDR_MM_DTYPE", "bf16")

# k-tile chunk sizes for the streamed x+w1 DMAs (sum == KT); small first
# chunks get the PE started early, large later ones amortize issue cost.
# Chunk i goes to HWDGE ring i%2 (sync/scalar) so both rings stream in
# parallel and arrival stays roughly k-ordered.
_CHUNKS = (1, 1, 2, 2, 3, 4, 5, 5, 5)
assert sum(_CHUNKS) == KT
# dummy matmuls issued while the first chunks are in flight: keeps the PE
# busy so the HAM clock-gate releases (1.2 -> 2.4 GHz) before real work.
_WARM_MMS = 8

_kernel_cache = {}

# test harness hooks: set _TRACE=True to profile; exec times land in
# _last_exec_ns (one entry per device launch).
_TRACE = False
_last_exec_ns = []
_last_results = []


def _build_mlp_kernel(cap: int, has_b1: bool, has_b2: bool):
    """Dense MLP on `cap` gathered rows (partition-major packed I/O)."""
    mm_dt = {"f32": F32, "f32r": mybir.dt.float32r,
             "bf16": mybir.dt.bfloat16, "f16": mybir.dt.float16}[_MM_DT_NAME]
    out_dt = F32 if _MM_DT_NAME in ("f32", "f32r") else mm_dt
    kt_x = KT + (1 if has_b1 else 0)   # augmented contraction tiles

    nc = bacc.Bacc("TRN2", target_bir_lowering=False, debug=False,
                   num_devices=N_CORES)
    # partition-major packed layouts: [p, k, ...] flattened on axis 1
    xp = nc.declare_dram_parameter("xp", [128, kt_x * cap], mm_dt, isOutput=False)
    w1p = nc.declare_dram_parameter("w1p", [128, kt_x * C_HID], mm_dt, isOutput=False)
    w2p = nc.declare_dram_parameter("w2p", [128, HT * C_OUT], mm_dt, isOutput=False)
    if has_b2:
        b2 = nc.declare_dram_parameter("b2", [1, C_OUT], F32, isOutput=False)
    outp = nc.declare_dram_parameter("outp", [128, OT * cap], out_dt, isOutput=True)

    # chunk boundaries (in k-tiles); b1's augmented tile rides the last chunk
    bounds = []
    a = 0
    for c in _CHUNKS:
        b = a + c
        if b == KT:
            b = kt_x
        bounds.append((a, b))
        a = b

    with TileContext(nc) as tc:
        with (
            tc.tile_pool(name="persist", bufs=1) as pp,
            tc.tile_pool(name="psum", bufs=1, space="PSUM") as psum_pool,
        ):
            x_all = pp.tile([128, kt_x * cap], mm_dt, name="x_all", tag="x")
            w1_all = pp.tile([128, kt_x * C_HID], mm_dt, name="w1_all", tag="w1")
            w2_all = pp.tile([128, HT * C_OUT], mm_dt, name="w2_all", tag="w2")
            h_sb = pp.tile([128, HT * cap], mm_dt, name="h_sb", tag="h")
            y_sb = pp.tile([128, OT * cap], out_dt, name="y_sb", tag="y")
            if has_b2:
                b2_sb = pp.tile([1, C_OUT], F32, name="b2_sb", tag="b2")
                ones_sb = pp.tile([1, cap], F32, name="ones_sb", tag="ones")

            # streamed ingest: each k-chunk carries its x slice then its w1
            # slice, chunks alternate between the two HWDGE rings
            # (sync/scalar) so both stream in parallel and arrival stays
            # k-ordered.  w2 rides the scalar ring at the end (needed only
            # at GEMM2 time, ~25 us in).
            rings = (nc.sync, nc.scalar)
            for ci, (ka, kb) in enumerate(bounds):
                eng = rings[ci % 2]
                eng.dma_start(out=x_all[:, ka * cap:kb * cap],
                              in_=xp[:, ka * cap:kb * cap])
                eng.dma_start(out=w1_all[:, ka * C_HID:kb * C_HID],
                              in_=w1p[:, ka * C_HID:kb * C_HID])
            nc.scalar.dma_start(out=w2_all[:], in_=w2p[:])
            if has_b2:
                nc.scalar.dma_start(out=b2_sb[:], in_=b2[:])
                nc.gpsimd.memset(ones_sb[:], 1.0)

            # HAM warmup: dummy matmuls on a zeroed tile run while the first
            # chunk DMAs are still in flight, so the clock-gate is already
            # released when real matmuls start.
            if _WARM_MMS:
                warm_sb = pp.tile([128, 512], mm_dt, name="warm_sb", tag="warm")
                warm_ps = psum_pool.tile([128, 512], F32, name="warm_ps",
                                         tag="warm_ps")
                nc.vector.memset(warm_sb[:], 0.0)
                for _ in range(_WARM_MMS):
                    nc.tensor.matmul(warm_ps[:], lhsT=warm_sb[:, :128],
                                     rhs=warm_sb[:], start=True, stop=True)

            # GEMM1: k-outer / h-inner; 6 PSUM banks accumulate concurrently.
            ps1 = [psum_pool.tile([128, cap], F32, name=f"ps1_{h}", tag=f"ps1_{h}")
                   for h in range(HT)]
            for k in range(kt_x):
                for h in range(HT):
                    nc.tensor.matmul(
                        ps1[h][:],
                        lhsT=w1_all[:, k * C_HID + h * 128:k * C_HID + (h + 1) * 128],
                        rhs=x_all[:, k * cap:(k + 1) * cap],
                        start=(k == 0),
                        stop=(k == kt_x - 1),
                    )
            for h in range(HT):
                nc.scalar.activation(h_sb[:, h * cap:(h + 1) * cap], ps1[h][:],
                                     AF.Silu)

            # GEMM2 (+ optional bias via K=1 ones-row matmul), h-outer so
            # each h-chunk is consumed as soon as its silu completes.
            # ps2 tiles tag-alias ps1[0..2] (freed once their silu reads).
            ps2 = [psum_pool.tile([128, cap], F32, name=f"ps2_{c}", tag=f"ps1_{c}")
                   for c in range(OT)]
            for h in range(HT):
                for c in range(OT):
                    nc.tensor.matmul(
                        ps2[c][:],
                        lhsT=w2_all[:, h * C_OUT + c * 128:h * C_OUT + (c + 1) * 128],
                        rhs=h_sb[:, h * cap:(h + 1) * cap],
                        start=(h == 0),
                        stop=(h == HT - 1) and not has_b2,
                    )
            for c in range(OT):
                if has_b2:
                    nc.tensor.matmul(
                        ps2[c][:],
                        lhsT=b2_sb[:, c * 128:(c + 1) * 128],
                        rhs=ones_sb[:],
                        start=False,
                        stop=True,
                    )
                # final copies on the vector engine so they don't queue
                # behind the silus on the scalar engine; each c-tile's
                # store DMA issues as soon as its copy lands
                nc.vector.tensor_scalar_add(y_sb[:, c * cap:(c + 1) * cap],
                                            ps2[c][:], 0.0)
                rings[c % 2].dma_start(out=outp[:, c * cap:(c + 1) * cap],
                                       in_=y_sb[:, c * cap:(c + 1) * cap])
    nc.compile()
    return nc


def _get_kernel(cap: int, has_b1: bool, has_b2: bool):
    key = (cap, has_b1, has_b2, _MM_DT_NAME)
    if key not in _kernel_cache:
        _kernel_cache[key] = _build_mlp_kernel(cap, has_b1, has_b2)
    return _kernel_cache[key]


def _alignment_indices(text_mask, chain_type_ids, cdr_region_type_ids,
                       boltz_chain_type, boltz_region_type):
    """All (b, text_src, protein_dst) triples, reference semantics."""
    tm = text_mask.astype(bool)
    bs, srcs, dsts = [], [], []
    for b in range(B):
        for ct in CHAIN_TYPES:
            for rt in CDR_TYPES:
                tmask = (chain_type_ids[b] == ct) & (cdr_region_type_ids[b] == rt) & tm[b]
                pmask = (boltz_chain_type[b] == ct) & (boltz_region_type[b] == rt)
                ti = np.nonzero(tmask)[0]
                pi = np.nonzero(pmask)[0]
                k = min(ti.shape[0], pi.shape[0])
                if k:
                    bs.append(np.full(k, b, np.int64))
                    srcs.append(ti[:k])
                    dsts.append(pi[:k])
    if not bs:
        z = np.zeros(0, np.int64)
        return z, z, z
    return np.concatenate(bs), np.concatenate(srcs), np.concatenate(dsts)


def _pack_pm(arr_t, kt, width):
    """[kt*128, width] -> partition-major [128, kt*width]."""
    return np.ascontiguousarray(
        arr_t.reshape(kt, 128, width).transpose(1, 0, 2).reshape(128, kt * width))


def kernel(text_conditioning, text_mask, chain_type_ids, cdr_region_type_ids,
           boltz_chain_type, boltz_region_type, W1, b1, W2, b2, scale):
    text_conditioning = np.asarray(text_conditioning, np.float32)
    W1 = np.asarray(W1, np.float32)
    b1v = np.asarray(b1, np.float32).reshape(-1)
    W2 = np.asarray(W2, np.float32)
    b2v = np.asarray(b2, np.float32).reshape(-1)
    scale_v = np.float32(np.asarray(scale).reshape(-1)[0])

    all_b, all_src, all_dst = _alignment_indices(
        np.asarray(text_mask), np.asarray(chain_type_ids),
        np.asarray(cdr_region_type_ids), np.asarray(boltz_chain_type),
        np.asarray(boltz_region_type))

    result = np.zeros((B, N_TOKEN, C_OUT), np.float32)
    nr = all_b.shape[0]
    if nr == 0:
        return result

    has_b1 = bool(b1v.any())
    b2s = b2v * scale_v
    has_b2 = bool(b2s.any())

    npdt = {"f32": np.float32, "f32r": np.float32,
            "bf16": ml_dtypes.bfloat16, "f16": np.float16}[_MM_DT_NAME]
    np_out_dt = np.float32 if _MM_DT_NAME in ("f32", "f32r") else npdt
    kt_x = KT + (1 if has_b1 else 0)

    # scale folds into the second layer
    w1T = np.ascontiguousarray(W1.T)                    # [3584, 768]
    if has_b1:
        aug = np.zeros((128, C_HID), np.float32)
        aug[0] = b1v
        w1T = np.concatenate([w1T, aug], axis=0)        # [3712, 768]
    w1p = _pack_pm(w1T, kt_x, C_HID).astype(npdt)
    w2T = np.ascontiguousarray((W2 * scale_v).T)        # [768, 384]
    w2p = _pack_pm(w2T, HT, C_OUT).astype(npdt)

    x_rows = text_conditioning[all_b, all_src, :]       # [nr, 3584]

    per_launch_cap = 512
    launch_rows = N_CORES * per_launch_cap
    y_rows = np.empty((nr, C_OUT), np.float32)

    for lo in range(0, nr, launch_rows):
        hi = min(nr, lo + launch_rows)
        n = hi - lo
        per_core = -(-n // N_CORES)
        cap = min(per_launch_cap, max(256, -(-per_core // 8) * 8))
        nc = _get_kernel(cap, has_b1, has_b2)
        in_maps = []
        bounds = []
        for c in range(N_CORES):
            a = lo + c * cap
            z = min(hi, a + cap)
            a = min(a, z)
            bounds.append((a, z))
            xT = np.zeros((kt_x * 128, cap), np.float32)
            if z > a:
                xT[:C_TEXT, :z - a] = x_rows[a:z].T
                if has_b1:
                    xT[C_TEXT, :z - a] = 1.0
            m = {"xp": _pack_pm(xT, kt_x, cap).astype(npdt),
                 "w1p": w1p, "w2p": w2p}
            if has_b2:
                m["b2"] = b2s.reshape(1, -1)
            in_maps.append(m)
        res = run_bass_kernel_spmd(nc, in_maps, list(range(N_CORES)),
                                   trace=_TRACE)
        if _TRACE:
            _last_exec_ns.append(res.exec_time_ns)
            _last_results.append(res)
        for c, (a, z) in enumerate(bounds):
            if z > a:
                o = res.results[c]["outp"]               # [128, OT*cap]
                yT = o.reshape(128, OT, cap).transpose(1, 0, 2).reshape(C_OUT, cap)
                y_rows[a:z] = yT[:, :z - a].T.astype(np.float32)

    result[all_b, all_dst, :] = y_rows
    return result


# revision 20
# speedup vs baseline: 1.0894x; 1.0894x over previous
"""CDR-aligned conditioner kernel for Trainium2 (8 NeuronCores).

Strategy
--------
The reference projects every text token through a 2-layer MLP
(3584 -> 768 -> SiLU -> 384) and then, per (chain_type, cdr_type) pair,
copies the k-th masked text row to the k-th masked protein position.
Only ~2450 of the 16384 text rows are ever read, so:

1. (host) compute the aligned (batch, text_src, protein_dst) triples
   with cheap integer ops — exactly the reference's cumsum/rank logic;
2. (host) gather just those text rows;
3. (device, 8 cores data-parallel over rows) dense MLP on the gathered
   rows:  Y^T = (scale*W2) @ silu(W1 @ X^T + b1) + scale*b2;
4. (host) scatter the projected rows into the zero-initialized output.

Device kernel notes (v2 — DMA-issue-bound fix):
- the v1 kernel issued 62 per-tile DMAs; each DMA_DIRECT2D occupies the
  sync engine ~650 ns generating descriptors, so the whole kernel was
  paced by DMA *issue* (~40 us serial), not HBM bandwidth.  v2 packs
  every operand partition-major in DRAM ([128, bytes] with large
  per-partition contiguous runs) so a handful of big DMAs move the same
  data: x on the scalar-engine HWDGE ring, w1 on the sync ring,
  chunked k-major so the GEMM1 k-loop streams as chunks land.
- all matmul operands are bf16 (full PE rate, half the HBM traffic of
  f32r; ~3e-3 max rel err vs the fp32 reference, tolerance is 2e-2).
- GEMM1 runs k-outer/h-inner with 6 PSUM banks accumulating; GEMM2
  h-outer/c-inner reuses 3 of those banks (tag-aliased) as silu frees
  them.  LDWEIGHTS is hidden by the PE's reorder window; warm matmuls
  stream at ~N/2.4GHz.
- output is written bf16 and upcast on host (halves the tail DMA).
"""

import os
import sys

sys.path.insert(0, "/opt/trn_rl_repo")

import ml_dtypes
import numpy as np

import concourse.bass as bass
import concourse.mybir as mybir
from concourse import bacc
from concourse.bass_utils import run_bass_kernel_spmd
from concourse.tile import TileContext

# Problem constants (hardcoded per contract)
B, L_TEXT, N_TOKEN = 8, 2048, 2048
C_TEXT, C_OUT = 3584, 384
C_HID = C_OUT * 2
CHAIN_TYPES = (1, 2)
CDR_TYPES = (2, 4, 6)
N_CORES = 8

KT = C_TEXT // 128   # 28 k-tiles (contraction of GEMM1)
HT = C_HID // 128    # 6 h-tiles
OT = C_OUT // 128    # 3 out-tiles

F32 = mybir.dt.float32
AF = mybir.ActivationFunctionType

# matmul element type: bf16 (default), float32r, or float32 (exact)
_MM_DT_NAME = os.environ.get("CDR_MM_DTYPE", "bf16")

# k-tile chunk sizes for the streamed x+w1 DMAs (sum == KT); small first
# chunks get the PE started early, large later ones amortize issue cost.
# All chunks ride ONE HWDGE ring (sync) so arrival is strictly k-ordered —
# splitting across rings lets later packets interleave ahead and the
# per-DMA completion (max over 16 engine slices) straggles by ~3 us.
_CHUNKS = (1, 1, 2, 3, 4, 5, 6, 6)
assert sum(_CHUNKS) == KT
# dummy matmuls issued while the first chunks are in flight: keeps the PE
# busy so the HAM clock-gate releases (1.2 -> 2.4 GHz) before real work.
_WARM_MMS = 8

_kernel_cache = {}

# test harness hooks: set _TRACE=True to profile; exec times land in
# _last_exec_ns (one entry per device launch).
_TRACE = False
_last_exec_ns = []
_last_results = []


def _build_mlp_kernel(cap: int, has_b1: bool, has_b2: bool):
    """Dense MLP on `cap` gathered rows (partition-major packed I/O)."""
    mm_dt = {"f32": F32, "f32r": mybir.dt.float32r,
             "bf16": mybir.dt.bfloat16, "f16": mybir.dt.float16}[_MM_DT_NAME]
    out_dt = F32 if _MM_DT_NAME in ("f32", "f32r") else mm_dt
    kt_x = KT + (1 if has_b1 else 0)   # augmented contraction tiles

    nc = bacc.Bacc("TRN2", target_bir_lowering=False, debug=False,
                   num_devices=N_CORES)
    # single combined ingest stream, partition-major: per k-tile a block of
    # [x_k (cap) | w1_k (C_HID)] elements per partition, then w2 as the
    # final block.  One DRAM param -> one ring -> arrival order == k order.
    BK = cap + C_HID
    w2_off = kt_x * BK
    total = w2_off + HT * C_OUT
    xw = nc.declare_dram_parameter("xw", [128, total], mm_dt, isOutput=False)
    if has_b2:
        b2 = nc.declare_dram_parameter("b2", [1, C_OUT], F32, isOutput=False)
    outp = nc.declare_dram_parameter("outp", [128, OT * cap], out_dt, isOutput=True)

    # chunk boundaries (in k-tiles); b1's augmented tile rides the last chunk
    bounds = []
    a = 0
    for c in _CHUNKS:
        b = a + c
        if b == KT:
            b = kt_x
        bounds.append((a, b))
        a = b

    with TileContext(nc) as tc:
        with (
            tc.tile_pool(name="persist", bufs=1) as pp,
            tc.tile_pool(name="psum", bufs=1, space="PSUM") as psum_pool,
        ):
            xw_all = pp.tile([128, total], mm_dt, name="xw_all", tag="xw")
            h_sb = pp.tile([128, HT * cap], mm_dt, name="h_sb", tag="h")
            y_sb = pp.tile([128, OT * cap], out_dt, name="y_sb", tag="y")
            if has_b2:
                b2_sb = pp.tile([1, C_OUT], F32, name="b2_sb", tag="b2")
                ones_sb = pp.tile([1, cap], F32, name="ones_sb", tag="ones")

            # streamed ingest: one DMA per k-chunk on the sync ring, strictly
            # k-ordered; w2 block last (needed only at GEMM2 time).
            for ka, kb in bounds:
                nc.sync.dma_start(out=xw_all[:, ka * BK:kb * BK],
                                  in_=xw[:, ka * BK:kb * BK])
            nc.sync.dma_start(out=xw_all[:, w2_off:total],
                              in_=xw[:, w2_off:total])
            if has_b2:
                nc.scalar.dma_start(out=b2_sb[:], in_=b2[:])
                nc.gpsimd.memset(ones_sb[:], 1.0)

            # HAM warmup: dummy matmuls on a zeroed tile run while the first
            # chunk DMAs are still in flight, so the clock-gate is already
            # released when real matmuls start.
            if _WARM_MMS:
                warm_sb = pp.tile([128, 512], mm_dt, name="warm_sb", tag="warm")
                warm_ps = psum_pool.tile([128, 512], F32, name="warm_ps",
                                         tag="warm_ps")
                nc.vector.memset(warm_sb[:], 0.0)
                for _ in range(_WARM_MMS):
                    nc.tensor.matmul(warm_ps[:], lhsT=warm_sb[:, :128],
                                     rhs=warm_sb[:], start=True, stop=True)

            # GEMM1: k-outer / h-inner; 6 PSUM banks accumulate concurrently.
            ps1 = [psum_pool.tile([128, cap], F32, name=f"ps1_{h}", tag=f"ps1_{h}")
                   for h in range(HT)]
            for k in range(kt_x):
                for h in range(HT):
                    nc.tensor.matmul(
                        ps1[h][:],
                        lhsT=xw_all[:, k * BK + cap + h * 128:
                                    k * BK + cap + (h + 1) * 128],
                        rhs=xw_all[:, k * BK:k * BK + cap],
                        start=(k == 0),
                        stop=(k == kt_x - 1),
                    )
            for h in range(HT):
                nc.scalar.activation(h_sb[:, h * cap:(h + 1) * cap], ps1[h][:],
                                     AF.Silu)

            # GEMM2 (+ optional bias via K=1 ones-row matmul), h-outer so
            # each h-chunk is consumed as soon as its silu completes.
            # ps2 tiles tag-alias ps1[0..2] (freed once their silu reads).
            ps2 = [psum_pool.tile([128, cap], F32, name=f"ps2_{c}", tag=f"ps1_{c}")
                   for c in range(OT)]
            for h in range(HT):
                for c in range(OT):
                    nc.tensor.matmul(
                        ps2[c][:],
                        lhsT=xw_all[:, w2_off + h * C_OUT + c * 128:
                                    w2_off + h * C_OUT + (c + 1) * 128],
                        rhs=h_sb[:, h * cap:(h + 1) * cap],
                        start=(h == 0),
                        stop=(h == HT - 1) and not has_b2,
                    )
            for c in range(OT):
                if has_b2:
                    nc.tensor.matmul(
                        ps2[c][:],
                        lhsT=b2_sb[:, c * 128:(c + 1) * 128],
                        rhs=ones_sb[:],
                        start=False,
                        stop=True,
                    )
                # final copies on the vector engine so they don't queue
                # behind the silus on the scalar engine; each c-tile's
                # store DMA issues as soon as its copy lands
                nc.vector.tensor_scalar_add(y_sb[:, c * cap:(c + 1) * cap],
                                            ps2[c][:], 0.0)
                nc.scalar.dma_start(out=outp[:, c * cap:(c + 1) * cap],
                                    in_=y_sb[:, c * cap:(c + 1) * cap])
    nc.compile()
    return nc


def _get_kernel(cap: int, has_b1: bool, has_b2: bool):
    key = (cap, has_b1, has_b2, _MM_DT_NAME)
    if key not in _kernel_cache:
        _kernel_cache[key] = _build_mlp_kernel(cap, has_b1, has_b2)
    return _kernel_cache[key]


def _alignment_indices(text_mask, chain_type_ids, cdr_region_type_ids,
                       boltz_chain_type, boltz_region_type):
    """All (b, text_src, protein_dst) triples, reference semantics."""
    tm = text_mask.astype(bool)
    bs, srcs, dsts = [], [], []
    for b in range(B):
        for ct in CHAIN_TYPES:
            for rt in CDR_TYPES:
                tmask = (chain_type_ids[b] == ct) & (cdr_region_type_ids[b] == rt) & tm[b]
                pmask = (boltz_chain_type[b] == ct) & (boltz_region_type[b] == rt)
                ti = np.nonzero(tmask)[0]
                pi = np.nonzero(pmask)[0]
                k = min(ti.shape[0], pi.shape[0])
                if k:
                    bs.append(np.full(k, b, np.int64))
                    srcs.append(ti[:k])
                    dsts.append(pi[:k])
    if not bs:
        z = np.zeros(0, np.int64)
        return z, z, z
    return np.concatenate(bs), np.concatenate(srcs), np.concatenate(dsts)


def _pack_xw(xT, w1T, w2T, kt_x, cap, npdt):
    """Combined partition-major ingest stream.

    Per k-tile block: [x_k (cap) | w1_k (C_HID)] per partition; w2 packed
    [128, HT*C_OUT] appended as the final block.
    """
    xb = xT.reshape(kt_x, 128, cap)
    wb = w1T.reshape(kt_x, 128, C_HID)
    kblocks = np.concatenate([xb, wb], axis=2)           # [kt_x, 128, BK]
    flat = kblocks.transpose(1, 0, 2).reshape(128, -1)
    w2b = w2T.reshape(HT, 128, C_OUT).transpose(1, 0, 2).reshape(128, -1)
    return np.ascontiguousarray(
        np.concatenate([flat, w2b], axis=1)).astype(npdt)


def kernel(text_conditioning, text_mask, chain_type_ids, cdr_region_type_ids,
           boltz_chain_type, boltz_region_type, W1, b1, W2, b2, scale):
    text_conditioning = np.asarray(text_conditioning, np.float32)
    W1 = np.asarray(W1, np.float32)
    b1v = np.asarray(b1, np.float32).reshape(-1)
    W2 = np.asarray(W2, np.float32)
    b2v = np.asarray(b2, np.float32).reshape(-1)
    scale_v = np.float32(np.asarray(scale).reshape(-1)[0])

    all_b, all_src, all_dst = _alignment_indices(
        np.asarray(text_mask), np.asarray(chain_type_ids),
        np.asarray(cdr_region_type_ids), np.asarray(boltz_chain_type),
        np.asarray(boltz_region_type))

    result = np.zeros((B, N_TOKEN, C_OUT), np.float32)
    nr = all_b.shape[0]
    if nr == 0:
        return result

    has_b1 = bool(b1v.any())
    b2s = b2v * scale_v
    has_b2 = bool(b2s.any())

    npdt = {"f32": np.float32, "f32r": np.float32,
            "bf16": ml_dtypes.bfloat16, "f16": np.float16}[_MM_DT_NAME]
    np_out_dt = np.float32 if _MM_DT_NAME in ("f32", "f32r") else npdt
    kt_x = KT + (1 if has_b1 else 0)

    # scale folds into the second layer
    w1T = np.ascontiguousarray(W1.T)                    # [3584, 768]
    if has_b1:
        aug = np.zeros((128, C_HID), np.float32)
        aug[0] = b1v
        w1T = np.concatenate([w1T, aug], axis=0)        # [3712, 768]
    w2T = np.ascontiguousarray((W2 * scale_v).T)        # [768, 384]

    x_rows = text_conditioning[all_b, all_src, :]       # [nr, 3584]

    per_launch_cap = 512
    launch_rows = N_CORES * per_launch_cap
    y_rows = np.empty((nr, C_OUT), np.float32)

    for lo in range(0, nr, launch_rows):
        hi = min(nr, lo + launch_rows)
        n = hi - lo
        per_core = -(-n // N_CORES)
        cap = min(per_launch_cap, max(256, -(-per_core // 8) * 8))
        nc = _get_kernel(cap, has_b1, has_b2)
        in_maps = []
        bounds = []
        for c in range(N_CORES):
            a = lo + c * cap
            z = min(hi, a + cap)
            a = min(a, z)
            bounds.append((a, z))
            xT = np.zeros((kt_x * 128, cap), np.float32)
            if z > a:
                xT[:C_TEXT, :z - a] = x_rows[a:z].T
                if has_b1:
                    xT[C_TEXT, :z - a] = 1.0
            m = {"xw": _pack_xw(xT, w1T, w2T, kt_x, cap, npdt)}
            if has_b2:
                m["b2"] = b2s.reshape(1, -1)
            in_maps.append(m)
        res = run_bass_kernel_spmd(nc, in_maps, list(range(N_CORES)),
                                   trace=_TRACE)
        if _TRACE:
            _last_exec_ns.append(res.exec_time_ns)
            _last_results.append(res)
        for c, (a, z) in enumerate(bounds):
            if z > a:
                o = res.results[c]["outp"]               # [128, OT*cap]
                yT = o.reshape(128, OT, cap).transpose(1, 0, 2).reshape(C_OUT, cap)
                y_rows[a:z] = yT[:, :z - a].T.astype(np.float32)

    result[all_b, all_dst, :] = y_rows
    return result
